# revision 1
# baseline (speedup 1.0000x reference)
"""Trainium2 Bass kernel for MixtralBlockSparseTop2MLP grouped-GEMM MoE.

Problem: 4096 rows (sorted by expert), 8 experts, hidden=1024, ffn=3584.
  out[r] = silu(x[r] @ W1g[e(r)]) * (x[r] @ W1u[e(r)]) @ W2[e(r)]

Sharding: tensor-parallel over the ffn dimension. Each of the 8 cores gets
a 448-channel slice of every expert's gate/up/down weights and computes a
partial output for ALL 4096 rows; the host sums the 8 partials. All cores
run the identical program (segment structure baked from rows_for_experts at
call time), so one SPMD NEFF serves all 8 cores with per-core weight data.

Compute dtype: bf16 matmul inputs with fp32 PSUM accumulation (fp32 matmul
is 4x slower on TRN2). Measured end-to-end rel err vs fp32 reference ~4e-3.
"""

import os
import sys

sys.path.insert(0, "/opt/trn_rl_repo")

import numpy as np
import ml_dtypes

E, R, H, F = 8, 1024 * 4, 1024, 3584
FC = F // 8          # 448 ffn channels per core
FCP = 512            # per-core ffn padded to 4 k-tiles of 128 for gemm2
NCH = 512            # row-chunk (gemm1 moving free dim; one PSUM bank per gate/up)
P = 128

BF16 = ml_dtypes.bfloat16

# test.py introspection: last BassKernelResults from run_bass_kernel_spmd
LAST_RESULT = None

_PROGRAM_CACHE = {}


def _segments(rows_for_experts):
    """[(expert, row_start, n_rows)] for experts with n_rows > 0."""
    segs = []
    r0 = 0
    for e in range(E):
        n = int(rows_for_experts[e])
        if n > 0:
            segs.append((e, r0, n))
        r0 += n
    # largest segment first (amortizes the split prologue weight load),
    # smallest last (shortens the end-of-kernel pipeline drain).
    segs.sort(key=lambda s: -s[2])
    return segs


def _chunk_list(segments):
    """[(expert, row_start, nch)] in program iteration order."""
    out = []
    for (e, r0, n_e) in segments:
        for c0 in range(0, n_e, NCH):
            out.append((e, r0 + c0, min(NCH, n_e - c0)))
    return out


def _build_program(segments, act_mode="silu"):
    import concourse.mybir as mybir
    import concourse.tile as tile
    from concourse import bacc

    dt = mybir.dt
    nc = bacc.Bacc(None, target_bir_lowering=False, debug=False)

    n_chunks = len(_chunk_list(segments))
    xT = nc.declare_dram_parameter(
        "xT", [n_chunks, P, H // P, NCH], dt.bfloat16, isOutput=False
    )
    w1 = nc.declare_dram_parameter("w1c", [E, P, H // P, 2 * FC], dt.bfloat16, isOutput=False)
    w2 = nc.declare_dram_parameter("w2c", [E, P, FCP // P, H], dt.bfloat16, isOutput=False)
    outp = nc.declare_dram_parameter(
        "outp", [n_chunks, P, NCH // P, H], dt.bfloat16, isOutput=True
    )

    KO = H // P        # 8 k-tiles for gemm1
    K2 = FCP // P      # 4 k-tiles for gemm2
    silu = mybir.ActivationFunctionType.Silu
    sigmoid = mybir.ActivationFunctionType.Sigmoid

    with tile.TileContext(nc) as tc:
        with (
            tc.tile_pool(name="w1p", bufs=4) as w1p,
            tc.tile_pool(name="w2p", bufs=3) as w2p,
            tc.tile_pool(name="xp", bufs=6) as xp,
            tc.tile_pool(name="apool", bufs=2) as apool,
            tc.tile_pool(name="a3pool", bufs=1) as a3pool,
            tc.tile_pool(name="opool", bufs=4) as opool,
            tc.tile_pool(name="hps", bufs=6, space="PSUM") as hps,
            tc.tile_pool(name="ops", bufs=2, space="PSUM") as ops,
        ):
            # a tile 3 holds only unit 6 in rows 0:64; rows 64:128 must be
            # exact zeros (they multiply the zero-padded w2 rows). Two
            # persistent ping-pong tiles, zeroed once.
            a3_tiles = [
                a3pool.tile([P, NCH], dt.bfloat16, tag=f"a3_{i}", name=f"a3_{i}")
                for i in range(2)
            ]
            for t3 in a3_tiles:
                nc.vector.memset(t3[:], 0.0)

            chunk_idx = 0
            first = True
            pending_gemm2 = None
            for (e, r0, n_e) in segments:
                w1sb = w1p.tile([P, KO, 2 * FC], dt.bfloat16, tag="w1sb")
                w2sb = w2p.tile([P, K2, H], dt.bfloat16, tag="w2sb")
                if first:
                    # prologue: load x chunk 0 + per-k w1 slices so the first
                    # matmuls start as soon as their own slice lands.
                    xsb0 = xp.tile([P, KO, NCH], dt.bfloat16, tag="xsb")
                    for k in range(KO):
                        nc.gpsimd.dma_start(xsb0[:, k, :], xT[0, :, k, :])
                        nc.sync.dma_start(w1sb[:, k, :], w1[e, :, k, :])
                    first = False
                else:
                    xsb0 = None
                    for k in range(KO):
                        nc.sync.dma_start(w1sb[:, k, :], w1[e, :, k, :])
                nc.sync.dma_start(w2sb[:], w2[e])

                for c0 in range(0, n_e, NCH):
                    nch = min(NCH, n_e - c0)
                    r = r0 + c0

                    if xsb0 is not None:
                        xsb, xsb0 = xsb0, None
                    else:
                        xsb = xp.tile([P, KO, NCH], dt.bfloat16, tag="xsb")
                        nc.gpsimd.dma_start(xsb[:], xT[chunk_idx])

                    # gemm1: 7 packed m-slices [gate_u(64) | up_u(64)];
                    # psum_u partitions 0:64 = gate, 64:128 = up.
                    # silu via ACT into a 64-row tmp, then DVE cross-base
                    # multiply into the packed a k-tiles.
                    a_tiles = [
                        apool.tile([P, NCH], dt.bfloat16, tag=f"a{j}", name=f"a{j}")
                        if j < 3
                        else a3_tiles[chunk_idx % 2]
                        for j in range(4)
                    ]
                    for u in (6, 0, 1, 2, 3, 4, 5):
                        hu_ps = hps.tile([P, NCH], dt.float32, tag="h", name=f"h{u}")
                        for k in range(KO):
                            nc.tensor.matmul(
                                hu_ps[:, :nch],
                                w1sb[:, k, P * u : P * u + P],
                                xsb[:, k, :nch],
                                start=(k == 0),
                                stop=(k == KO - 1),
                            )
                        stmp = apool.tile([64, NCH], dt.bfloat16, tag="stmp", name="stmp")
                        if act_mode == "silu":
                            nc.scalar.activation(
                                stmp[:, :nch], hu_ps[0:64, :nch], silu
                            )
                        else:  # silu(g) = g * sigmoid(g); CoreSim lacks Silu
                            nc.scalar.activation(
                                stmp[:, :nch], hu_ps[0:64, :nch], sigmoid
                            )
                            nc.vector.tensor_mul(
                                stmp[:, :nch], stmp[:, :nch], hu_ps[0:64, :nch]
                            )
                        lo = 64 * (u % 2)
                        nc.vector.tensor_mul(
                            a_tiles[u // 2][lo : lo + 64, :nch],
                            stmp[:, :nch],
                            hu_ps[64:128, :nch],
                        )

                    # gemm2 (emitted one chunk behind gemm1 so the PE never
                    # waits on this chunk's silu/mul chain): stage the whole
                    # chunk in one [P, 4, H] tile, store with one contiguous
                    # chunk-major DMA; the host unpacks valid rows.
                    def gemm2(
                        ci=chunk_idx, nch=nch, a_tiles=a_tiles, w2sb=w2sb
                    ):
                        # rotate output stores over the three DMA queues;
                        # keep the last two chunks off the SWDGE (gpsimd)
                        # ring so its end-of-kernel drain is empty.
                        dma_eng = (nc.sync, nc.gpsimd)[ci % 2]
                        osb = opool.tile(
                            [P, NCH // P, H], dt.bfloat16, tag="osb", name="osb"
                        )
                        for s0 in range(0, nch, P):
                            rows = min(P, nch - s0)
                            s = s0 // P
                            otiles = [
                                ops.tile([P, 512], dt.float32, tag="o", name=f"o{h2}")
                                for h2 in range(H // 512)
                            ]
                            for k in range(K2):
                                for h2 in range(H // 512):
                                    nc.tensor.matmul(
                                        otiles[h2][:rows, :],
                                        a_tiles[k][:, s0 : s0 + rows],
                                        w2sb[:, k, 512 * h2 : 512 * h2 + 512],
                                        start=(k == 0),
                                        stop=(k == K2 - 1),
                                    )
                            for h2 in range(H // 512):
                                nc.vector.tensor_copy(
                                    osb[:rows, s, 512 * h2 : 512 * h2 + 512],
                                    otiles[h2][:rows, :],
                                )
                        sf = nch // P
                        rem = nch - P * sf
                        if ci >= n_chunks - 3:
                            # tail chunks: halve each store across both rings
                            # so the end-of-kernel write drain is short.
                            if sf:
                                h_sf = (sf + 1) // 2
                                nc.sync.dma_start(
                                    outp[ci][:, :h_sf, :], osb[:, :h_sf, :]
                                )
                                if sf > h_sf:
                                    nc.gpsimd.dma_start(
                                        outp[ci][:, h_sf:sf, :],
                                        osb[:, h_sf:sf, :],
                                    )
                            if rem:
                                h_r = (rem + 1) // 2
                                nc.sync.dma_start(
                                    outp[ci][:h_r, sf, :], osb[:h_r, sf, :]
                                )
                                nc.gpsimd.dma_start(
                                    outp[ci][h_r:rem, sf, :],
                                    osb[h_r:rem, sf, :],
                                )
                        else:
                            if sf:
                                dma_eng.dma_start(
                                    outp[ci][:, :sf, :], osb[:, :sf, :]
                                )
                            if rem:
                                dma_eng.dma_start(
                                    outp[ci][:rem, sf, :], osb[:rem, sf, :]
                                )

                    if pending_gemm2 is not None:
                        pending_gemm2()
                    pending_gemm2 = gemm2
                    chunk_idx += 1
            pending_gemm2()

    nc.compile()
    return nc


def _prepare_inputs(hidden_states, w1, w2, chunks):
    """Host-side shard/layout/cast. Returns (xT, [w1c per core], [w2c per core])."""
    x = np.asarray(hidden_states, dtype=np.float32)
    w1 = np.asarray(w1, dtype=np.float32)
    w2 = np.asarray(w2, dtype=np.float32)

    xb = x.astype(BF16)          # [R, H]
    w1b = w1.astype(BF16)        # [E, H, 2F]
    w2b = w2.astype(BF16)        # [E, F, H]

    # chunk-major x: xT[c, p, ko, j] = x[chunk_c_row0 + j, 128*ko + p]
    # (8KB contiguous per partition per chunk -> full-rate DMA)
    xTflat = np.ascontiguousarray(xb.T.reshape(H // P, P, R).transpose(1, 0, 2))
    xT = np.zeros((len(chunks), P, H // P, NCH), dtype=BF16)
    for ci, (_, r, nch) in enumerate(chunks):
        xT[ci, :, :, :nch] = xTflat[:, :, r : r + nch]

    w1cs, w2cs = [], []
    for c in range(8):
        gate = w1b[:, :, c * FC : (c + 1) * FC]
        up = w1b[:, :, F + c * FC : F + (c + 1) * FC]
        # interleave 64-channel blocks: [G0|U0|G1|U1|...|G6|U6] so each
        # 128-column m-slice u packs gate_u in psum partitions 0:64 and
        # up_u in 64:128.
        w1cat = np.ascontiguousarray(
            np.stack(
                [gate.reshape(E, H, FC // 64, 64), up.reshape(E, H, FC // 64, 64)],
                axis=3,
            ).reshape(E, H, 2 * FC)
        )
        w1c = np.ascontiguousarray(
            w1cat.reshape(E, H // P, P, 2 * FC).transpose(0, 2, 1, 3)
        )
        w2pad = np.zeros((E, FCP, H), dtype=BF16)
        w2pad[:, :FC, :] = w2b[:, c * FC : (c + 1) * FC, :]
        w2c = np.ascontiguousarray(
            w2pad.reshape(E, FCP // P, P, H).transpose(0, 2, 1, 3)
        )
        w1cs.append(w1c)
        w2cs.append(w2c)
    return xT, w1cs, w2cs


def kernel(hidden_states, w1, w2, rows_for_experts):
    global LAST_RESULT
    from concourse.bass_utils import run_bass_kernel_spmd

    segs = _segments(np.asarray(rows_for_experts))
    if not segs:
        return np.zeros((R, H), dtype=np.float32)
    key = tuple(segs)
    nc = _PROGRAM_CACHE.get(key)
    if nc is None:
        nc = _build_program(segs)
        _PROGRAM_CACHE[key] = nc

    xT, w1cs, w2cs = _prepare_inputs(hidden_states, w1, w2, _chunk_list(segs))
    in_maps = [
        {"xT": xT, "w1c": w1cs[c], "w2c": w2cs[c]} for c in range(8)
    ]
    res = run_bass_kernel_spmd(nc, in_maps, core_ids=list(range(8)))
    LAST_RESULT = res

    chunks = _chunk_list(segs)
    acc = np.zeros((R, H), dtype=np.float32)
    for c in range(8):
        blocks = res.results[c]["outp"]  # [n_chunks, P, NCH//P, H] bf16
        for ci, (_, r, nch) in enumerate(chunks):
            rowsmaj = blocks[ci].transpose(1, 0, 2).reshape(NCH, H)[:nch]
            acc[r : r + nch] += rowsmaj.astype(np.float32)
    return acc

